# revision 1
# baseline (speedup 1.0000x reference)
"""Causal attention (B=4, S=4096, D=768) on 8 Trainium2 NeuronCores.

Sharding: zigzag query-strip packing. Each batch b is handled by two cores
(roles): role 0 owns query strips {0,2,5,7}, role 1 owns {1,3,4,6} (strips of
512 rows). Both roles run the IDENTICAL program (SPMD): 4 query supers of 512
rows, with per-super key-block loop bounds TSLOT=(8,16,24,32) 128-row blocks.
Strip->slot assignment is chosen so each role's strip needs <= the slot bound;
the overshoot plus the causal diagonal are killed by additive mask planes
(host-precomputed per role, supplied as input data). Softmax uses no
max-subtraction (scores/sqrt(D) ~ N(0,1); exp is safe in fp32); the
denominator comes free from a ones-column appended to V. Host prep: cast to
bf16, transpose x, pack query strips (layout-only work; all FLOPs on device).
"""

import math

import numpy as np
import ml_dtypes

P = 128
NEG = -1e9
bf16 = ml_dtypes.bfloat16

# Full-size problem geometry (hardcoded; kernel.py must be self-contained).
B, S, D = 4, 4096, 768
SUP = 512
NSLOT = 4
NQ = NSLOT * SUP
TSLOT = (8, 16, 24, 32)
MASK_KB = 8
ROLE_STRIPS = ((0, 2, 5, 7), (1, 3, 4, 6))
N_CORES = 8


def build_program(S, D, SUP, TSLOT, MASK_KB, out_dtype_np=np.float32):
    """Build the single SPMD Bass program (one core's view).

    Inputs (per core): xkT bf16 [D,S], xqT bf16 [D,NQ], wq/wk/wv bf16 [D,D],
    rmask f32 [NSLOT, P, MASK_KB*SUP]. Output: out f32 [NQ, D] (packed rows).
    """
    import concourse.bass as bass
    import concourse.tile as tile
    import concourse.mybir as mybir
    from concourse import bacc

    DC = D // P
    NSLOT_ = len(TSLOT)
    NQ_ = NSLOT_ * SUP
    NKB = S // P
    ED = D + 1  # V gets a ones column appended -> denominator for free
    # free-dim splits for the PV matmul over the augmented [0, ED) columns
    osplits = []
    pos = 0
    while pos < ED:
        osplits.append((pos, min(pos + 512, ED)))
        pos = min(pos + 512, ED)
    # splits of [0, D) for the V projection
    vsplits = []
    pos = 0
    while pos < D:
        vsplits.append((pos, min(pos + 512, D)))
        pos = min(pos + 512, D)
    SCALE = 1.0 / math.sqrt(float(D))
    f32 = mybir.dt.float32
    b16 = mybir.dt.bfloat16

    nc = bacc.Bacc("TRN2", target_bir_lowering=False, debug=False)

    xkT = nc.dram_tensor("xkT", [D, S], b16, kind="ExternalInput").ap()
    xqT = nc.dram_tensor("xqT", [D, NQ_], b16, kind="ExternalInput").ap()
    whs = {
        n: nc.dram_tensor(n, [D, D], b16, kind="ExternalInput").ap()
        for n in ("wq", "wk", "wv")
    }
    rmask = nc.dram_tensor(
        "rmask", [NSLOT_, P, MASK_KB * SUP], f32, kind="ExternalInput"
    ).ap()
    out = nc.dram_tensor(
        "out", [NQ_, D], mybir.dt.from_np(np.dtype(out_dtype_np)), kind="ExternalOutput"
    ).ap()

    xkT_r = xkT.rearrange("(c p) s -> p c s", p=P)
    xqT_r = xqT.rearrange("(c p) s -> p c s", p=P)

    with tile.TileContext(nc) as tc:
        with (
            tc.tile_pool(name="persist", bufs=1) as persist,
            tc.tile_pool(name="xstage", bufs=3) as xstage,
        ):
            # persistent SBUF tensors
            KT = persist.tile([P, DC, S], b16, name="KT")        # K^T, d on partitions
            QT = persist.tile([P, DC, NQ_], b16, name="QT")      # Q^T
            V = persist.tile([P, NKB, ED], b16, name="V")       # V by key-block, +ones col
            nc.vector.memset(V[:, :, D:ED], 1.0)

            # ---------------- phase 1: projections ----------------
            with (
                tc.tile_pool(name="wpool", bufs=1) as wpool,
                tc.tile_pool(name="ppsum", bufs=4, space="PSUM") as ppsum,
            ):
                W = {}
                for n in ("wq", "wk", "wv"):
                    W[n] = wpool.tile([P, DC, D], b16, tag=n, name=n)
                    nc.sync.dma_start(W[n], whs[n].rearrange("(c p) e -> p c e", p=P))

                def project_chunk(xT_t, s0, width, kt_dst, with_v):
                    """xT_t: [P, DC, width] bf16 chunk of x^T starting at col s0."""
                    for do in range(DC):
                        ps = ppsum.tile([P, 512], f32, tag="proj", name="proj_ps")[:, :width]
                        wsrc = W["wk"] if kt_dst is KT else W["wq"]
                        for dc in range(DC):
                            nc.tensor.matmul(
                                ps,
                                lhsT=wsrc[:, dc, do * P:(do + 1) * P],
                                rhs=xT_t[:, dc, :width],
                                start=(dc == 0),
                                stop=(dc == DC - 1),
                            )
                        nc.any.tensor_copy(
                            out=kt_dst[:, do, s0:s0 + width], in_=ps
                        )
                    if with_v:
                        for sb in range(width // P):
                            kb = (s0 + sb * P) // P
                            for (e0, e1) in vsplits:
                                ps = ppsum.tile([P, 512], f32, tag="projv", name="projv_ps")[:, :e1 - e0]
                                for dc in range(DC):
                                    nc.tensor.matmul(
                                        ps,
                                        lhsT=xT_t[:, dc, sb * P:(sb + 1) * P],
                                        rhs=W["wv"][:, dc, e0:e1],
                                        start=(dc == 0),
                                        stop=(dc == DC - 1),
                                    )
                                nc.any.tensor_copy(
                                    out=V[:, kb, e0:e1], in_=ps
                                )

                CHUNK = 512
                for ch in range(S // CHUNK):
                    xT_t = xstage.tile([P, DC, CHUNK], b16, tag="xk", name="xk_t")
                    nc.sync.dma_start(
                        xT_t, xkT_r[:, :, ch * CHUNK:(ch + 1) * CHUNK]
                    )
                    project_chunk(xT_t, ch * CHUNK, CHUNK, KT, with_v=True)
                for ch in range(NQ_ // CHUNK):
                    xT_t = xstage.tile([P, DC, CHUNK], b16, tag="xq", name="xq_t")
                    nc.sync.dma_start(
                        xT_t, xqT_r[:, :, ch * CHUNK:(ch + 1) * CHUNK]
                    )
                    project_chunk(xT_t, ch * CHUNK, CHUNK, QT, with_v=False)

            # ---------------- phase 2: attention ----------------
            with (
                tc.tile_pool(name="expp", bufs=1) as expp,
                tc.tile_pool(name="mpool", bufs=2) as mpool,
                tc.tile_pool(name="opool", bufs=2) as opool,
                tc.tile_pool(name="spsum", bufs=2, space="PSUM") as spsum,
                tc.tile_pool(name="opsumA", bufs=2, space="PSUM") as opsumA,
                tc.tile_pool(name="opsumB", bufs=2, space="PSUM") as opsumB,
            ):
                Tmax = max(TSLOT)
                expT = expp.tile([P, Tmax, SUP], b16, name="expT")
                for t in range(NSLOT_):
                    T = TSLOT[t]
                    q0 = t * SUP
                    # scores + exp for all key blocks of this super
                    for kb in range(T):
                        ps = spsum.tile([P, SUP], f32, tag="sc", name="sc_ps")
                        for dc in range(DC):
                            nc.tensor.matmul(
                                ps,
                                lhsT=KT[:, dc, kb * P:(kb + 1) * P],
                                rhs=QT[:, dc, q0:q0 + SUP],
                                start=(dc == 0),
                                stop=(dc == DC - 1),
                            )
                        if kb >= T - MASK_KB:
                            kbi = kb - (T - MASK_KB)
                            m = mpool.tile([P, SUP], f32, tag="m", name="m_t")
                            nc.sync.dma_start(
                                m, rmask[t, :, kbi * SUP:(kbi + 1) * SUP]
                            )
                            nc.vector.tensor_add(ps, ps, m)
                        nc.scalar.activation(
                            expT[:, kb, :], ps,
                            mybir.ActivationFunctionType.Exp, scale=SCALE,
                        )
                    # out = (expT)^T @ [V | 1] per 128-row query slice
                    for sl in range(SUP // P):
                        bound = min(T, T - (SUP // P) + 1 + sl)
                        pss = []
                        for (e0, e1) in osplits:
                            pss.append(
                                opsumA.tile([P, 512], f32, tag="oA", name="oA_ps")[:, :e1 - e0]
                                if e0 == 0
                                else opsumB.tile([P, ED - 512], f32, tag="oB", name="oB_ps")
                            )
                        for kb in range(bound):
                            for (e0, e1), ps_o in zip(osplits, pss):
                                nc.tensor.matmul(
                                    ps_o,
                                    lhsT=expT[:, kb, sl * P:(sl + 1) * P],
                                    rhs=V[:, kb, e0:e1],
                                    start=(kb == 0),
                                    stop=(kb == bound - 1),
                                )
                        recip = opool.tile([P, 1], f32, tag="recip", name="recip_t")
                        last = pss[-1]
                        nc.vector.reciprocal(recip, last[:, last.shape[-1] - 1:])
                        ot = opool.tile([P, D], mybir.dt.from_np(np.dtype(out_dtype_np)), tag="ot", name="ot_t")
                        for (e0, e1), ps_o in zip(osplits, pss):
                            hi = min(e1, D)
                            nc.vector.tensor_scalar_mul(
                                ot[:, e0:hi], ps_o[:, :hi - e0], recip
                            )
                        nc.sync.dma_start(
                            out[q0 + sl * P: q0 + (sl + 1) * P, :], ot
                        )

    nc.compile()
    return nc


def make_rmask(role_strips, TSLOT, SUP, MASK_KB):
    nslot = len(TSLOT)
    m = np.zeros((nslot, P, MASK_KB * SUP), np.float32)
    i = np.arange(P)[:, None]
    j = np.arange(SUP)[None, :]
    for t in range(nslot):
        q0 = SUP * role_strips[t]
        T = TSLOT[t]
        for kbi in range(MASK_KB):
            k0 = P * (T - MASK_KB + kbi)
            m[t, :, kbi * SUP:(kbi + 1) * SUP] = np.where(
                q0 + j >= k0 + i, 0.0, NEG
            )
    return m


_nc_cache = {}
last_run = None


def _get_nc():
    key = (S, D, SUP, TSLOT, MASK_KB)
    if key not in _nc_cache:
        _nc_cache[key] = build_program(S, D, SUP, TSLOT, MASK_KB)
    return _nc_cache[key]



def make_in_maps(x, w_b):
    rmasks = [make_rmask(ROLE_STRIPS[r], TSLOT, SUP, MASK_KB) for r in range(2)]
    in_maps = []
    for c in range(N_CORES):
        b, role = c % B, c // B
        xb = x[b].astype(bf16)
        xq = np.concatenate(
            [xb[SUP * s:SUP * (s + 1)] for s in ROLE_STRIPS[role]], axis=0
        )
        in_maps.append({
            "xkT": np.ascontiguousarray(xb.T),
            "xqT": np.ascontiguousarray(xq.T),
            "rmask": rmasks[role],
            **w_b,
        })
    return in_maps


def kernel(x, Wq, Wk, Wv):
    from concourse import bass_utils

    x = np.asarray(x, dtype=np.float32)
    w_b = {n: np.asarray(w, np.float32).astype(bf16)
           for n, w in (("wq", Wq), ("wk", Wk), ("wv", Wv))}

    nc = _get_nc()

    in_maps = make_in_maps(x, w_b)

    global last_run
    last_run = bass_utils.run_bass_kernel_spmd(
        nc, in_maps, core_ids=list(range(N_CORES))
    )
    res = last_run.results

    out = np.empty((B, S, D), np.float32)
    for c in range(N_CORES):
        b, role = c % B, c // B
        packed = res[c]["out"]
        for t, s in enumerate(ROLE_STRIPS[role]):
            out[b, SUP * s:SUP * (s + 1)] = packed[SUP * t:SUP * (t + 1)]
    return out


if __name__ == "__main__":
    import reference

    inputs = {k: np.asarray(v) for k, v in reference.setup_inputs().items()}
    expected = np.asarray(reference.reference(**inputs))
    actual = kernel(**inputs)
    err = np.abs(actual - expected).max()
    print(f"absmax err: {err:.3e}  rel: {err / np.abs(expected).max():.3e}")



# revision 2
# speedup vs baseline: 1.2332x; 1.2332x over previous
"""Causal attention (B=4, S=4096, D=768) on 8 Trainium2 NeuronCores.

Sharding: zigzag query-strip packing (as baseline kernel.py) — each batch is
handled by two cores; role 0 owns query strips {0,2,5,7}, role 1 owns
{1,3,4,6} (strips of 512 rows); per-super key-block bounds TSLOT=(8,16,24,32).

Numerics: refined-fp8 DoubleRow matmuls. Every value a is split a = a8 + da
(both fp8e4m3, with power-of-2 prescales so residuals stay out of the fp8
subnormal floor), and products keep the three leading terms
(a8 b8 + da b8 + a8 db) — the dropped da*db term is O(1e-3) relative. Each
DoubleRow matmul packs two 128-deep contraction halves at 0.5 cycles/row:
  - projections: x (x8|dx, prescale 16) @ W (W8|dW, prescale 64), 9 DR MMs
    per 2 contraction blocks -> 0.75x bf16 cycles, better-than-bf16 accuracy.
  - scores QK: same 3-term scheme on (k8|dk)x(q8|dq) -> 0.75x cycles.
  - PV (slots 1-3): exp stored fp8 with offset c (exp(s*SCALE - c), offset
    cancels in num/den), V refined (V8|dV): per key-block pair, 2 DR MMs
    sharing one stationary (e8 pair) -> 0.5x cycles.
  - PV slot 0 (query rows < 1024, small-denominator rows): bf16 exp & V.
Softmax denominator comes free from a ones-column appended to V8 (dV ones
col = 0). Measured numpy end-to-end rel err ~3.4e-3 (budget 2e-2).
"""

import math

import numpy as np
import ml_dtypes

P = 128
NEG = -1e9
bf16 = ml_dtypes.bfloat16
f8np = ml_dtypes.float8_e4m3

B, S, D = 4, 4096, 768
DC = D // P
SUP = 512
NSLOT = 4
NQ = NSLOT * SUP
TSLOT = (8, 16, 24, 32)
MASK_KB = 8
ROLE_STRIPS = ((0, 2, 5, 7), (1, 3, 4, 6))
N_CORES = 8

XS, WS = 16.0, 64.0          # host prescales for x and W fp8 splits
INV = 1.0 / (XS * WS)        # 2**-10, exact
C_OFF = 2.0                  # exp offset for fp8 slots (1..3)
T0 = TSLOT[0]                # slot-0 key blocks (bf16 PV path)
VPAD = 784                   # fp8 V row pitch (769 used, %16==0)
V16PAD = 776                 # bf16 V row pitch for slot 0
OSPLITS = ((0, 512), (512, 769))
VSPLITS = ((0, 512), (512, 768))


def build_program(out_dtype_np=np.float32):
    import concourse.tile as tile
    import concourse.mybir as mybir
    from concourse import bacc

    NKB = S // P
    SCALE = 1.0 / math.sqrt(float(D))
    f32 = mybir.dt.float32
    b16 = mybir.dt.bfloat16
    f8 = mybir.dt.float8e4
    DR = mybir.MatmulPerfMode.DoubleRow
    MUL = mybir.AluOpType.mult
    SUB = mybir.AluOpType.subtract

    nc = bacc.Bacc("TRN2", target_bir_lowering=False, debug=False)

    xkc = nc.dram_tensor("xkc", [2 * D, S], f8, kind="ExternalInput").ap()
    xqc = nc.dram_tensor("xqc", [2 * D, NQ], f8, kind="ExternalInput").ap()
    whs = {
        n: nc.dram_tensor(n, [2 * D, D], f8, kind="ExternalInput").ap()
        for n in ("wq", "wk", "wv")
    }
    rmask = nc.dram_tensor(
        "rmask", [NSLOT, P, MASK_KB * SUP], f32, kind="ExternalInput"
    ).ap()
    out = nc.dram_tensor(
        "out", [NQ, D], mybir.dt.from_np(np.dtype(out_dtype_np)), kind="ExternalOutput"
    ).ap()

    xkc_r = xkc.rearrange("(c p) s -> p c s", p=P)   # c: 0..5 x8, 6..11 dx
    xqc_r = xqc.rearrange("(c p) s -> p c s", p=P)

    with tile.TileContext(nc) as tc:
        with (
            tc.tile_pool(name="persist", bufs=1) as persist,
            tc.tile_pool(name="xstage", bufs=3) as xstage,
        ):
            # persistent SBUF: fp8 splits, d(c)-blocks 0..5 = hi, 6..11 = lo
            KT = persist.tile([P, 2 * DC, S], f8, name="KT")
            QT = persist.tile([P, 2 * DC, NQ], f8, name="QT")
            V8 = persist.tile([P, NKB, VPAD], f8, name="V8")
            dV = persist.tile([P, NKB, VPAD], f8, name="dV")
            V16 = persist.tile([P, T0, V16PAD], b16, name="V16")
            nc.vector.memset(V8[:, :, D:D + 1], 1.0)
            nc.vector.memset(dV[:, :, D:D + 1], 0.0)
            nc.vector.memset(V16[:, :, D:D + 1], 1.0)

            # ---------------- phase 1: projections ----------------
            with (
                tc.tile_pool(name="wpool", bufs=1) as wpool,
                tc.tile_pool(name="ppsum", bufs=4, space="PSUM") as ppsum,
            ):
                W = {}
                for n in ("wq", "wk", "wv"):
                    W[n] = wpool.tile([P, 2 * DC, D], f8, tag=n, name=n)
                    nc.sync.dma_start(W[n], whs[n].rearrange("(c p) e -> p c e", p=P))

                def refined_group(ps, wt, wof, xt, xof, j, nj):
                    """9 DR MMs: A=(w8,x8) C=(w8,dx) B=(dw,x8) per dc-pair j.
                    wt/xt: [P, 12, *]; wof/xof: free-dim slices."""
                    w8 = wt[:, 2 * j:2 * j + 2, wof[0]:wof[1]]
                    dw = wt[:, 6 + 2 * j:8 + 2 * j, wof[0]:wof[1]]
                    x8 = xt[:, 2 * j:2 * j + 2, xof[0]:xof[1]]
                    dx = xt[:, 6 + 2 * j:8 + 2 * j, xof[0]:xof[1]]
                    nc.tensor.matmul(ps, lhsT=w8, rhs=x8, start=(j == 0),
                                     stop=False, perf_mode=DR)
                    nc.tensor.matmul(ps, lhsT=w8, rhs=dx, start=False,
                                     stop=False, perf_mode=DR)
                    nc.tensor.matmul(ps, lhsT=dw, rhs=x8, start=False,
                                     stop=(j == nj - 1), perf_mode=DR)

                def project_chunk(xc_t, s0, width, dst, with_v):
                    for do in range(DC):
                        ps = ppsum.tile([P, 512], f32, tag="proj",
                                        name="proj_ps")[:, :width]
                        for j in range(DC // 2):
                            refined_group(ps, W["wk" if dst is KT else "wq"],
                                          (do * P, (do + 1) * P),
                                          xc_t, (0, width), j, DC // 2)
                        hi = dst[:, do, s0:s0 + width]
                        nc.scalar.activation(
                            hi, ps, mybir.ActivationFunctionType.Copy,
                            scale=INV)
                        nc.vector.scalar_tensor_tensor(
                            dst[:, 6 + do, s0:s0 + width], ps, INV, hi,
                            op0=MUL, op1=SUB)
                    if with_v:
                        for sb in range(width // P):
                            kb = (s0 + sb * P) // P
                            for (e0, e1) in VSPLITS:
                                ps = ppsum.tile([P, 512], f32, tag="projv",
                                                name="projv_ps")[:, :e1 - e0]
                                for j in range(DC // 2):
                                    # stationary = x pairs, moving = W pairs
                                    xs8 = xc_t[:, 2 * j:2 * j + 2,
                                               sb * P:(sb + 1) * P]
                                    xsd = xc_t[:, 6 + 2 * j:8 + 2 * j,
                                               sb * P:(sb + 1) * P]
                                    w8 = W["wv"][:, 2 * j:2 * j + 2, e0:e1]
                                    wd = W["wv"][:, 6 + 2 * j:8 + 2 * j, e0:e1]
                                    nc.tensor.matmul(ps, lhsT=xs8, rhs=w8,
                                                     start=(j == 0),
                                                     stop=False, perf_mode=DR)
                                    nc.tensor.matmul(ps, lhsT=xs8, rhs=wd,
                                                     start=False, stop=False,
                                                     perf_mode=DR)
                                    nc.tensor.matmul(ps, lhsT=xsd, rhs=w8,
                                                     start=False,
                                                     stop=(j == DC // 2 - 1),
                                                     perf_mode=DR)
                                hi = V8[:, kb, e0:e1]
                                nc.scalar.activation(
                                    hi, ps, mybir.ActivationFunctionType.Copy,
                                    scale=INV)
                                nc.vector.scalar_tensor_tensor(
                                    dV[:, kb, e0:e1], ps, INV, hi,
                                    op0=MUL, op1=SUB)
                                if kb < T0:
                                    nc.scalar.activation(
                                        V16[:, kb, e0:e1], ps,
                                        mybir.ActivationFunctionType.Copy,
                                        scale=INV)

                CHUNK = 512
                for ch in range(S // CHUNK):
                    xc_t = xstage.tile([P, 2 * DC, CHUNK], f8, tag="xk",
                                       name="xk_t")
                    nc.sync.dma_start(
                        xc_t, xkc_r[:, :, ch * CHUNK:(ch + 1) * CHUNK])
                    project_chunk(xc_t, ch * CHUNK, CHUNK, KT, with_v=True)
                for ch in range(NQ // CHUNK):
                    xc_t = xstage.tile([P, 2 * DC, CHUNK], f8, tag="xq",
                                       name="xq_t")
                    nc.sync.dma_start(
                        xc_t, xqc_r[:, :, ch * CHUNK:(ch + 1) * CHUNK])
                    project_chunk(xc_t, ch * CHUNK, CHUNK, QT, with_v=False)

            # ---------------- phase 2: attention ----------------
            with (
                tc.tile_pool(name="expp", bufs=1) as expp,
                tc.tile_pool(name="mpool", bufs=2) as mpool,
                tc.tile_pool(name="opool", bufs=2) as opool,
                tc.tile_pool(name="spsum", bufs=2, space="PSUM") as spsum,
                tc.tile_pool(name="opsumA", bufs=2, space="PSUM") as opsumA,
                tc.tile_pool(name="opsumB", bufs=2, space="PSUM") as opsumB,
            ):
                Tmax = max(TSLOT)
                expT8 = expp.tile([P, Tmax, SUP], f8, name="expT8")
                expT16 = expp.tile([P, T0, SUP], b16, name="expT16")
                cbias = expp.tile([P, 1], f32, name="cbias")
                nc.vector.memset(cbias, -C_OFF)
                for t in range(NSLOT):
                    T = TSLOT[t]
                    q0 = t * SUP
                    for kb in range(T):
                        ps = spsum.tile([P, SUP], f32, tag="sc", name="sc_ps")
                        for j in range(DC // 2):
                            refined_group(ps, KT, (kb * P, (kb + 1) * P),
                                          QT, (q0, q0 + SUP), j, DC // 2)
                        if kb >= T - MASK_KB:
                            kbi = kb - (T - MASK_KB)
                            m = mpool.tile([P, SUP], f32, tag="m", name="m_t")
                            nc.sync.dma_start(
                                m, rmask[t, :, kbi * SUP:(kbi + 1) * SUP])
                            nc.vector.tensor_add(ps, ps, m)
                        if t == 0:
                            nc.scalar.activation(
                                expT16[:, kb, :], ps,
                                mybir.ActivationFunctionType.Exp, scale=SCALE)
                        else:
                            nc.scalar.activation(
                                expT8[:, kb, :], ps,
                                mybir.ActivationFunctionType.Exp,
                                scale=SCALE, bias=cbias)
                    for sl in range(SUP // P):
                        bound = min(T, T - (SUP // P) + 1 + sl)
                        psA = opsumA.tile([P, 512], f32, tag="oA", name="oA")
                        psB = opsumB.tile([P, 257], f32, tag="oB", name="oB")
                        pss = (psA, psB)
                        if t == 0:
                            for kb in range(bound):
                                for (e0, e1), ps_o in zip(OSPLITS, pss):
                                    nc.tensor.matmul(
                                        ps_o,
                                        lhsT=expT16[:, kb,
                                                    sl * P:(sl + 1) * P],
                                        rhs=V16[:, kb, e0:e1],
                                        start=(kb == 0),
                                        stop=(kb == bound - 1))
                        else:
                            nkbp = (bound + 1) // 2
                            for kbp in range(nkbp):
                                e8 = expT8[:, 2 * kbp:2 * kbp + 2,
                                           sl * P:(sl + 1) * P]
                                for (e0, e1), ps_o in zip(OSPLITS, pss):
                                    nc.tensor.matmul(
                                        ps_o, lhsT=e8,
                                        rhs=V8[:, 2 * kbp:2 * kbp + 2, e0:e1],
                                        start=(kbp == 0), stop=False,
                                        perf_mode=DR)
                                    nc.tensor.matmul(
                                        ps_o, lhsT=e8,
                                        rhs=dV[:, 2 * kbp:2 * kbp + 2, e0:e1],
                                        start=False, stop=(kbp == nkbp - 1),
                                        perf_mode=DR)
                        recip = opool.tile([P, 1], f32, tag="recip",
                                           name="recip_t")
                        nc.vector.reciprocal(recip, psB[:, 256:257])
                        ot = opool.tile([P, D],
                                        mybir.dt.from_np(np.dtype(out_dtype_np)),
                                        tag="ot", name="ot_t")
                        for (e0, e1), ps_o in zip(OSPLITS, pss):
                            hi = min(e1, D)
                            nc.vector.tensor_scalar_mul(
                                ot[:, e0:hi], ps_o[:, :hi - e0], recip)
                        nc.sync.dma_start(
                            out[q0 + sl * P: q0 + (sl + 1) * P, :], ot)

    nc.compile()
    return nc


def make_rmask(role_strips):
    m = np.zeros((NSLOT, P, MASK_KB * SUP), np.float32)
    i = np.arange(P)[:, None]
    j = np.arange(SUP)[None, :]
    for t in range(NSLOT):
        q0 = SUP * role_strips[t]
        T = TSLOT[t]
        for kbi in range(MASK_KB):
            k0 = P * (T - MASK_KB + kbi)
            m[t, :, kbi * SUP:(kbi + 1) * SUP] = np.where(
                q0 + j >= k0 + i, 0.0, NEG)
    return m


def split8_T(a):
    """fp8 split of a [rows, cols]: returns [2*cols, rows] (hi rows, lo rows)."""
    hi = a.astype(f8np)
    lo = (a - hi.astype(np.float32)).astype(f8np)
    return np.ascontiguousarray(
        np.concatenate([hi.T, lo.T], axis=0))


def split8_rows(a):
    """fp8 split of a [rows, cols]: returns [2*rows, cols] (hi rows, lo rows)."""
    hi = a.astype(f8np)
    lo = (a - hi.astype(np.float32)).astype(f8np)
    return np.ascontiguousarray(np.concatenate([hi, lo], axis=0))


_nc_cache = {}
last_run = None


def _get_nc():
    key = "fp8"
    if key not in _nc_cache:
        _nc_cache[key] = build_program()
    return _nc_cache[key]


def make_in_maps(x, w_c):
    rmasks = [make_rmask(ROLE_STRIPS[r]) for r in range(2)]
    in_maps = []
    for c in range(N_CORES):
        b, role = c % B, c // B
        xb = x[b] * XS
        xq = np.concatenate(
            [xb[SUP * s:SUP * (s + 1)] for s in ROLE_STRIPS[role]], axis=0)
        in_maps.append({
            "xkc": split8_T(xb),
            "xqc": split8_T(xq),
            "rmask": rmasks[role],
            **w_c,
        })
    return in_maps


def kernel(x, Wq, Wk, Wv):
    from concourse import bass_utils

    x = np.asarray(x, np.float32)
    w_c = {n: split8_rows(np.asarray(w, np.float32) * WS)
           for n, w in (("wq", Wq), ("wk", Wk), ("wv", Wv))}

    nc = _get_nc()
    in_maps = make_in_maps(x, w_c)

    global last_run
    last_run = bass_utils.run_bass_kernel_spmd(
        nc, in_maps, core_ids=list(range(N_CORES)))
    res = last_run.results

    out = np.empty((B, S, D), np.float32)
    for c in range(N_CORES):
        b, role = c % B, c // B
        packed = res[c]["out"]
        for t, s in enumerate(ROLE_STRIPS[role]):
            out[b, SUP * s:SUP * (s + 1)] = packed[SUP * t:SUP * (t + 1)]
    return out


if __name__ == "__main__":
    import reference

    inputs = {k: np.asarray(v) for k, v in reference.setup_inputs().items()}
    expected = np.asarray(reference.reference(**inputs))
    actual = kernel(**inputs)
    err = np.abs(actual - expected).max()
    print(f"absmax err: {err:.3e}  rel: {err / np.abs(expected).max():.3e}")
